# revision 18
# baseline (speedup 1.0000x reference)
"""Depth-warping layer for Trainium2 (Bass/Tile), 8-core data-parallel.

Strategy
--------
Pure data parallelism over the batch dim: each of the 8 NeuronCores
processes 2 of the 16 batch images end to end (no collectives).

Per batch image, on device:
  Phase A: compute d1_calc = W2z + depth2 * (m20*x + m21*y + m22) into
    DRAM, then per 128-row tile build a "quad table" row block
    J2[r, c, 0:4] = (I[rA,cA], I[rB,cA], I[rA,cB], I[rB,cB]) by
    interleaving 4 shifted copies in SBUF (Scalar engine) and writing ONE
    contiguous DMA per tile (a strided-write formulation costs 10.5M
    4-byte DMA packets = 32.5ms; this costs 128 x 20KB packets = 0.4ms).
  Phase B: streaming coordinate math (u2, v2, floor, clip, weights) with
    the exact reference op order (bit-exact u2/v2 so floor decisions
    match), then a hardware For_i loop of [128,1]-offset indirect DMA
    gathers (128 descriptors each, the only offset shape the SWDGE
    indirect ucode supports) fetching each pixel's 16-byte quad,
    round-robined over the 4 SWDGE queues, then the 4-tap combine.

The gather is the hard bound: the SWDGE ucode decodes ~1.05us per
indirect instruction (128 descriptors), serial on the Pool engine, so
~2.6M pixel-gathers per core cost ~21.5ms of Pool time. Everything else
is arranged to hide under it: the affine row-planes come from one shared
iota row times per-batch scalars (no per-batch row buffers, so batch 2's
phase A overlaps batch 1's phase B), the combine uses dedicated temps
(so tile t+1's coordinate math overlaps tile t's gathers), and the i32
scratch is sequenced x-side-then-y-side to halve its footprint.

Host does only the O(1) per-batch 3x3 matrix algebra and ships per-batch
scalars as small aux tensors (the NEFF is shared by all cores, so
per-batch constants must arrive as data, not compile-time immediates).
"""

import numpy as np

import concourse.bass as bass
import concourse.bacc as bacc
import concourse.mybir as mybir
from concourse.tile import TileContext
from concourse import bass_utils

B, H, W = 16, 1024, 1280
NCORES = 8
BPC = B // NCORES          # batches per core
HP = H + 1                 # J2 rows
WP = W + 1                 # J2 cols
NTILES = H // 128
UNROLL = 320               # gather columns per For_i iteration

F32 = mybir.dt.float32
I32 = mybir.dt.int32
OP = mybir.AluOpType


def _build_bass():
    nc = bacc.Bacc(target_bir_lowering=False, num_swdge_queues=4)

    d1 = nc.dram_tensor("d1", [BPC, H, W], F32, kind="ExternalInput")
    d2 = nc.dram_tensor("d2", [BPC, H, W], F32, kind="ExternalInput")
    # One shared iota row (x replicated to all partitions), batch-invariant
    urow = nc.dram_tensor("urow", [128, W], F32, kind="ExternalInput")
    # Per-tile per-partition columns [BPC, 128, NTILES]
    colA = nc.dram_tensor("colA", [BPC, 128, NTILES], F32, kind="ExternalInput")  # -M01*y
    colB = nc.dram_tensor("colB", [BPC, 128, NTILES], F32, kind="ExternalInput")  # -M11*y
    colC = nc.dram_tensor("colC", [BPC, 128, NTILES], F32, kind="ExternalInput")  # M21*y
    colG = nc.dram_tensor("colG", [BPC, 128, NTILES], F32, kind="ExternalInput")  # m2_21*y
    # Per-batch scalars replicated to [BPC, 128, 1]
    names = ["cwx", "cwy", "cwz", "cw2", "cA2", "cB2", "cC2", "cG2",
             "sA0", "sB0", "sC0", "sG0"]
    scal = {n: nc.dram_tensor(n, [BPC, 128, 1], F32, kind="ExternalInput")
            for n in names}
    out = nc.dram_tensor("out", [BPC, H, W], F32, kind="ExternalOutput")

    with TileContext(nc) as tc:
        with tc.tile_pool(name="dram", bufs=2, space="DRAM") as dpool, \
             tc.tile_pool(name="cst", bufs=1) as cpool, \
             tc.tile_pool(name="io", bufs=1) as iop, \
             tc.tile_pool(name="tmp", bufs=1) as tp, \
             tc.tile_pool(name="gat", bufs=1) as gp:

            urow_t = cpool.tile([128, W], F32, tag="urow")
            nc.sync.dma_start(out=urow_t[:], in_=urow[:])

            for lb in range(BPC):
                J2 = dpool.tile([HP, WP, 4], F32, tag="J2")

                st = {}
                for n in names:
                    st[n] = cpool.tile([128, 1], F32, tag=n, bufs=2, name=f"st_{n}")
                    nc.sync.dma_start(out=st[n][:], in_=scal[n][lb])
                colA_t = cpool.tile([128, NTILES], F32, tag="colA", bufs=2)
                colB_t = cpool.tile([128, NTILES], F32, tag="colB", bufs=2)
                colC_t = cpool.tile([128, NTILES], F32, tag="colC", bufs=2)
                colG_t = cpool.tile([128, NTILES], F32, tag="colG", bufs=2)
                nc.sync.dma_start(out=colA_t[:], in_=colA[lb])
                nc.sync.dma_start(out=colB_t[:], in_=colB[lb])
                nc.sync.dma_start(out=colC_t[:], in_=colC[lb])
                nc.sync.dma_start(out=colG_t[:], in_=colG[lb])

                def ts(dst, in0, s1, s2, o0, o1=None):
                    nc.vector.tensor_scalar(out=dst, in0=in0, scalar1=s1,
                                            scalar2=s2, op0=o0,
                                            **({"op1": o1} if o1 is not None else {}))

                # ---- Phase A: d1_calc for the whole image into DRAM ----
                d1cD = dpool.tile([H, W], F32, tag="d1cD")
                for t in range(NTILES):
                    y0 = 128 * t
                    d2t = iop.tile([128, W], F32, tag="d2t", bufs=2)
                    nc.sync.dma_start(out=d2t[:], in_=d2[lb, y0:y0 + 128, :])
                    # g = (m2_20*x + m2_21*y) + m2_22  (reference assoc order)
                    g = tp.tile([128, W], F32, tag="g")
                    ts(g[:], urow_t[:], st["sG0"][:, 0:1], None, OP.mult)
                    nc.vector.tensor_tensor(
                        out=g[:], in0=g[:],
                        in1=colG_t[:, t:t + 1].to_broadcast([128, W]), op=OP.add)
                    ts(g[:], g[:], st["cG2"][:, 0:1], None, OP.add)
                    # d1c = d2*g + W2z
                    d1cw = tp.tile([128, W], F32, tag="d1cw", bufs=2)
                    nc.vector.tensor_tensor(
                        out=d1cw[:], in0=d2t[:], in1=g[:], op=OP.mult)
                    ts(d1cw[:], d1cw[:], st["cw2"][:, 0:1], None, OP.add)
                    nc.sync.dma_start(out=d1cD[y0:y0 + 128, :], in_=d1cw[:])

                # J2 row 0: quad = (I[0,cA], I[0,cA], I[0,cB], I[0,cB])
                ld0 = iop.tile([128, W], F32, tag="d2t", bufs=2)
                nc.sync.dma_start(out=ld0[0:1, :], in_=d1cD[0:1, :])
                j2r0 = tp.tile([128, WP, 4], F32, tag="j2sb")
                nc.vector.memset(j2r0[0:1, 0, :], 0.0)
                nc.vector.memset(j2r0[0:1, W, :], 0.0)
                nc.scalar.copy(out=j2r0[0:1, 1:W, 0], in_=ld0[0:1, 0:W - 1])
                nc.scalar.copy(out=j2r0[0:1, 1:W, 1], in_=ld0[0:1, 0:W - 1])
                nc.scalar.copy(out=j2r0[0:1, 1:W, 2], in_=ld0[0:1, 1:W])
                nc.scalar.copy(out=j2r0[0:1, 1:W, 3], in_=ld0[0:1, 1:W])
                nc.sync.dma_start(out=J2[0:1, :, :], in_=j2r0[0:1, :, :])

                # Per tile t: J2 rows 128t+1 .. 128t+128.
                #   k=0,2 from I rows 128t..128t+127  (cur)
                #   k=1,3 from I rows 128t+1..128t+128 (nxt; row H -> H-1)
                for t in range(NTILES):
                    y0 = 128 * t
                    cur = iop.tile([128, W], F32, tag="curt")
                    nc.sync.dma_start(out=cur[:], in_=d1cD[y0:y0 + 128, :])
                    nxt = tp.tile([128, W], F32, tag="nxt")
                    if t < NTILES - 1:
                        nc.sync.dma_start(out=nxt[:], in_=d1cD[y0 + 1:y0 + 129, :])
                    else:
                        nc.sync.dma_start(out=nxt[0:127, :],
                                          in_=d1cD[y0 + 1:y0 + 128, :])
                        # J2 row H: rB = min(H, H-1) = H-1 -> last image row
                        nc.sync.dma_start(out=nxt[127:128, :],
                                          in_=d1cD[H - 1:H, :])
                    j2sb = tp.tile([128, WP, 4], F32, tag="j2sb")
                    nc.vector.memset(j2sb[:, 0, :], 0.0)
                    nc.vector.memset(j2sb[:, W, :], 0.0)
                    nc.scalar.copy(out=j2sb[:, 1:W, 0], in_=cur[:, 0:W - 1])
                    nc.scalar.copy(out=j2sb[:, 1:W, 1], in_=nxt[:, 0:W - 1])
                    nc.scalar.copy(out=j2sb[:, 1:W, 2], in_=cur[:, 1:W])
                    nc.scalar.copy(out=j2sb[:, 1:W, 3], in_=nxt[:, 1:W])
                    nc.sync.dma_start(
                        out=J2[128 * t + 1:128 * t + 129, :, :], in_=j2sb[:])

                J2flat = J2[:].rearrange("a b c -> (a b) c")

                # ---- Phase B ----
                for t in range(NTILES):
                    y0 = 128 * t
                    z1 = iop.tile([128, W], F32, tag="z1")
                    nc.sync.dma_start(out=z1[:], in_=d1[lb, y0:y0 + 128, :])

                    # f32 scratch fa..fe die before the gather; weights live
                    # until the combine (which has its own temps ca/cb so the
                    # next tile's math can overlap this tile's gathers).
                    fa = tp.tile([128, W], F32, tag="fa")
                    fb = tp.tile([128, W], F32, tag="fb")
                    fc = tp.tile([128, W], F32, tag="fc")
                    fd = tp.tile([128, W], F32, tag="fd")
                    fe = tp.tile([128, W], F32, tag="fe")
                    u2 = tp.tile([128, W], F32, tag="u2")
                    v2 = tp.tile([128, W], F32, tag="v2")
                    # A = (-M00*x + -M01*y) + -M02, reference assoc order
                    ts(fa[:], urow_t[:], st["sA0"][:, 0:1], None, OP.mult)
                    nc.vector.tensor_tensor(
                        out=fa[:], in0=fa[:],
                        in1=colA_t[:, t:t + 1].to_broadcast([128, W]), op=OP.add)
                    ts(fa[:], fa[:], st["cA2"][:, 0:1], None, OP.add)
                    ts(fb[:], urow_t[:], st["sB0"][:, 0:1], None, OP.mult)
                    nc.vector.tensor_tensor(
                        out=fb[:], in0=fb[:],
                        in1=colB_t[:, t:t + 1].to_broadcast([128, W]), op=OP.add)
                    ts(fb[:], fb[:], st["cB2"][:, 0:1], None, OP.add)
                    ts(fc[:], urow_t[:], st["sC0"][:, 0:1], None, OP.mult)
                    nc.vector.tensor_tensor(
                        out=fc[:], in0=fc[:],
                        in1=colC_t[:, t:t + 1].to_broadcast([128, W]), op=OP.add)
                    ts(fc[:], fc[:], st["cC2"][:, 0:1], None, OP.add)
                    # z2 = z1*C + Wv2
                    zd = tp.tile([128, W], F32, tag="zd")
                    nc.vector.tensor_tensor(out=zd[:], in0=z1[:], in1=fc[:], op=OP.mult)
                    ts(zd[:], zd[:], st["cwz"][:, 0:1], None, OP.add)
                    nc.vector.reciprocal(out=fe[:], in_=zd[:])          # r0
                    nc.vector.tensor_tensor(out=fd[:], in0=zd[:], in1=fe[:], op=OP.mult)
                    ts(fd[:], fd[:], 2.0, None, OP.subtract)            # z2*r0-2
                    nc.vector.tensor_tensor(out=fe[:], in0=fe[:], in1=fd[:], op=OP.mult)  # q=-1/z2
                    # nU = z1*A + (-Wv0)  (A,Wv negated on host); u2' = nU*q
                    nU = tp.tile([128, W], F32, tag="nU")
                    nc.vector.tensor_tensor(out=nU[:], in0=z1[:], in1=fa[:], op=OP.mult)
                    ts(nU[:], nU[:], st["cwx"][:, 0:1], None, OP.add)
                    nc.vector.tensor_tensor(out=u2[:], in0=nU[:], in1=fe[:], op=OP.mult)
                    # quotient correction: e = z2*u2' + nU; u2 = u2' + e*q
                    nc.vector.tensor_tensor(out=fd[:], in0=zd[:], in1=u2[:], op=OP.mult)
                    nc.vector.tensor_tensor(out=fd[:], in0=fd[:], in1=nU[:], op=OP.add)
                    nc.vector.tensor_tensor(out=fd[:], in0=fd[:], in1=fe[:], op=OP.mult)
                    nc.vector.tensor_tensor(out=u2[:], in0=u2[:], in1=fd[:], op=OP.add)
                    nc.vector.tensor_tensor(out=nU[:], in0=z1[:], in1=fb[:], op=OP.mult)
                    ts(nU[:], nU[:], st["cwy"][:, 0:1], None, OP.add)
                    nc.vector.tensor_tensor(out=v2[:], in0=nU[:], in1=fe[:], op=OP.mult)
                    nc.vector.tensor_tensor(out=fd[:], in0=zd[:], in1=v2[:], op=OP.mult)
                    nc.vector.tensor_tensor(out=fd[:], in0=fd[:], in1=nU[:], op=OP.add)
                    nc.vector.tensor_tensor(out=fd[:], in0=fd[:], in1=fe[:], op=OP.mult)
                    nc.vector.tensor_tensor(out=v2[:], in0=v2[:], in1=fd[:], op=OP.add)

                    wxa = tp.tile([128, W], F32, tag="wxa")
                    wxc = tp.tile([128, W], F32, tag="wxc")
                    wya = tp.tile([128, W], F32, tag="wya")
                    wyb = tp.tile([128, W], F32, tag="wyb")
                    ia = tp.tile([128, W], I32, tag="ia")
                    ib2 = tp.tile([128, W], I32, tag="ib2")
                    icc = tp.tile([128, W], I32, tag="icc")
                    idt = tp.tile([128, W], I32, tag="idt")
                    flat = tp.tile([128, W], I32, tag="flat")

                    # --- x side: exact floor, clips, weights ---
                    # floor: t = cvt(u2); t -= (cvt_f32(t) > u2)
                    nc.vector.tensor_copy(out=ia[:], in_=u2[:])
                    nc.vector.tensor_copy(out=fa[:], in_=ia[:])
                    nc.vector.tensor_tensor(out=fc[:], in0=fa[:], in1=u2[:], op=OP.is_gt)
                    nc.vector.tensor_copy(out=idt[:], in_=fc[:])
                    nc.vector.tensor_tensor(out=ia[:], in0=ia[:], in1=idt[:], op=OP.subtract)
                    # m1 = max(x0+1,0); x1c = min(m1,W-1); cc = min(m1,W); x0c
                    ts(idt[:], ia[:], 1, 0, OP.add, OP.max)
                    ts(ib2[:], idt[:], W - 1, None, OP.min)             # x1c
                    ts(icc[:], idt[:], W, None, OP.min)                 # cc (to flat)
                    ts(ia[:], ia[:], 0, W - 1, OP.max, OP.min)          # x0c in place
                    nc.vector.tensor_copy(out=fa[:], in_=ib2[:])        # x1f
                    nc.vector.tensor_tensor(out=wxa[:], in0=fa[:], in1=u2[:], op=OP.subtract)
                    nc.vector.tensor_copy(out=fa[:], in_=ia[:])         # x0f
                    nc.vector.tensor_tensor(out=wxc[:], in0=u2[:], in1=fa[:], op=OP.subtract)

                    # --- y side (reuses ia/ib2/idt) ---
                    nc.vector.tensor_copy(out=ia[:], in_=v2[:])
                    nc.vector.tensor_copy(out=fb[:], in_=ia[:])
                    nc.vector.tensor_tensor(out=fc[:], in0=fb[:], in1=v2[:], op=OP.is_gt)
                    nc.vector.tensor_copy(out=idt[:], in_=fc[:])
                    nc.vector.tensor_tensor(out=ia[:], in0=ia[:], in1=idt[:], op=OP.subtract)
                    ts(idt[:], ia[:], 1, 0, OP.add, OP.max)
                    ts(ib2[:], idt[:], H, None, OP.min)                 # rr (to flat)
                    ts(idt[:], idt[:], H - 1, None, OP.min)             # y1c in place
                    ts(ia[:], ia[:], 0, H - 1, OP.max, OP.min)          # y0c
                    nc.vector.tensor_copy(out=fb[:], in_=idt[:])        # y1f
                    nc.vector.tensor_tensor(out=wya[:], in0=fb[:], in1=v2[:], op=OP.subtract)
                    nc.vector.tensor_copy(out=fb[:], in_=ia[:])         # y0f
                    nc.vector.tensor_tensor(out=wyb[:], in0=v2[:], in1=fb[:], op=OP.subtract)
                    nc.vector.scalar_tensor_tensor(
                        out=flat[:], in0=ib2[:], scalar=WP, in1=icc[:],
                        op0=OP.mult, op1=OP.add)

                    # gather loop: 128 descriptors per indirect DMA
                    gq = gp.tile([128, W, 4], F32, tag="gq")
                    ib = gp.tile([128, UNROLL], I32, tag="ib", bufs=2)
                    gb = gp.tile([128, UNROLL, 4], F32, tag="gb", bufs=2)
                    with tc.For_i(0, W, UNROLL) as iv:
                        nc.vector.tensor_copy(out=ib[:], in_=flat[:, bass.ds(iv, UNROLL)])
                        for j in range(UNROLL):
                            inst = nc.gpsimd.indirect_dma_start(
                                out=gb[:, j, :], out_offset=None,
                                in_=J2flat,
                                in_offset=bass.IndirectOffsetOnAxis(ap=ib[:, j:j + 1], axis=0),
                            )
                            inst.ins.queue = f"qPoolDynamic{(j // 4) % 4 or ''}"
                        nc.vector.tensor_copy(out=gq[:, bass.ds(iv, UNROLL), :], in_=gb[:])

                    # combine, reference order:
                    # ((wa*Ia + wb*Ib) + wc*Ic) + wd*Id, wa = wxa*wya etc.
                    # Dedicated temps ca/cb + in-place weight products keep
                    # fa..fe free so tile t+1's math overlaps these ops.
                    ca = tp.tile([128, W], F32, tag="ca")
                    cb = tp.tile([128, W], F32, tag="cb")
                    ot = iop.tile([128, W], F32, tag="ot")
                    nc.vector.tensor_tensor(out=ca[:], in0=wxa[:], in1=wyb[:], op=OP.mult)   # wb
                    nc.vector.tensor_tensor(out=wxa[:], in0=wxa[:], in1=wya[:], op=OP.mult)  # wa
                    nc.vector.tensor_tensor(out=cb[:], in0=wxc[:], in1=wya[:], op=OP.mult)   # wc
                    nc.vector.tensor_tensor(out=wxc[:], in0=wxc[:], in1=wyb[:], op=OP.mult)  # wd
                    nc.vector.tensor_tensor(out=ot[:], in0=wxa[:], in1=gq[:, :, 0], op=OP.mult)
                    nc.vector.tensor_tensor(out=wxa[:], in0=ca[:], in1=gq[:, :, 1], op=OP.mult)
                    nc.vector.tensor_tensor(out=ot[:], in0=ot[:], in1=wxa[:], op=OP.add)
                    nc.vector.tensor_tensor(out=wxa[:], in0=cb[:], in1=gq[:, :, 2], op=OP.mult)
                    nc.vector.tensor_tensor(out=ot[:], in0=ot[:], in1=wxa[:], op=OP.add)
                    nc.vector.tensor_tensor(out=wxa[:], in0=wxc[:], in1=gq[:, :, 3], op=OP.mult)
                    nc.vector.tensor_tensor(out=ot[:], in0=ot[:], in1=wxa[:], op=OP.add)
                    nc.sync.dma_start(out=out[lb, y0:y0 + 128, :], in_=ot[:])

    nc.finalize()
    return nc


def _host_aux(translation, rotation, intrinsic):
    """Per-batch coefficient tensors (f32, mirroring reference order of ops)."""
    K = intrinsic.astype(np.float32)
    Kinv = np.linalg.inv(K).astype(np.float32)
    R = rotation.astype(np.float32)
    t = translation.astype(np.float32)
    nb = R.shape[0]
    temp = np.einsum('ij,bkj->bik', K, R).astype(np.float32)
    Wv = np.einsum('bij,bjk->bik', temp, -t).astype(np.float32)     # [nb,3,1]
    M = np.einsum('bij,jk->bik', temp, Kinv).astype(np.float32)     # [nb,3,3]
    W2 = np.einsum('ij,bjk->bik', K, t).astype(np.float32)
    M2 = np.einsum('bij,jk->bik', np.einsum('ij,bjk->bik', K, R), Kinv).astype(np.float32)

    y = np.arange(H, dtype=np.float32)
    ycols = y.reshape(NTILES, 128).T                                # [128, NTILES]

    aux = {}
    aux["colA"] = -(M[:, 0, 1][:, None, None] * ycols[None])
    aux["colB"] = -(M[:, 1, 1][:, None, None] * ycols[None])
    aux["colC"] = (M[:, 2, 1][:, None, None] * ycols[None])
    aux["colG"] = (M2[:, 2, 1][:, None, None] * ycols[None])
    ones = np.ones((nb, 128, 1), np.float32)
    aux["cwx"] = -Wv[:, 0, 0][:, None, None] * ones
    aux["cwy"] = -Wv[:, 1, 0][:, None, None] * ones
    aux["cwz"] = Wv[:, 2, 0][:, None, None] * ones
    aux["cw2"] = W2[:, 2, 0][:, None, None] * ones
    aux["cA2"] = -M[:, 0, 2][:, None, None] * ones
    aux["cB2"] = -M[:, 1, 2][:, None, None] * ones
    aux["cC2"] = M[:, 2, 2][:, None, None] * ones
    aux["cG2"] = M2[:, 2, 2][:, None, None] * ones
    aux["sA0"] = -M[:, 0, 0][:, None, None] * ones
    aux["sB0"] = -M[:, 1, 0][:, None, None] * ones
    aux["sC0"] = M[:, 2, 0][:, None, None] * ones
    aux["sG0"] = M2[:, 2, 0][:, None, None] * ones
    for k in aux:
        aux[k] = np.ascontiguousarray(aux[k].astype(np.float32))
    return aux


_NC_CACHE = {}


def kernel(depth_map_1, depth_map_2, translation, rotation, intrinsic):
    d1 = np.ascontiguousarray(np.asarray(depth_map_1, dtype=np.float32)[..., 0])
    d2 = np.ascontiguousarray(np.asarray(depth_map_2, dtype=np.float32)[..., 0])
    t = np.asarray(translation, dtype=np.float32)
    R = np.asarray(rotation, dtype=np.float32)
    K = np.asarray(intrinsic, dtype=np.float32)

    if "nc" not in _NC_CACHE:
        _NC_CACHE["nc"] = _build_bass()
    nc = _NC_CACHE["nc"]

    aux = _host_aux(t, R, K)
    urow = np.ascontiguousarray(
        np.tile(np.arange(W, dtype=np.float32)[None, :], (128, 1)))

    in_maps = []
    for c in range(NCORES):
        sl = slice(c * BPC, (c + 1) * BPC)
        m = {"d1": d1[sl], "d2": d2[sl], "urow": urow}
        for k, v in aux.items():
            m[k] = v[sl]
        in_maps.append(m)

    res = bass_utils.run_bass_kernel_spmd(nc, in_maps, core_ids=list(range(NCORES)))
    out = np.empty((B, H, W, 1), np.float32)
    for c in range(NCORES):
        out[c * BPC:(c + 1) * BPC, :, :, 0] = res.results[c]["out"]
    return out


# revision 19
# speedup vs baseline: 1.0115x; 1.0115x over previous
"""Depth-warping layer for Trainium2 (Bass/Tile), 8-core data-parallel.

Strategy
--------
Pure data parallelism over the batch dim: each of the 8 NeuronCores
processes 2 of the 16 batch images end to end (no collectives).

Per batch image, on device:
  Phase A: compute d1_calc = W2z + depth2 * (m20*x + m21*y + m22) into
    DRAM, then per 128-row tile build a "quad table" row block
    J2[r, c, 0:4] = (I[rA,cA], I[rB,cA], I[rA,cB], I[rB,cB]) by
    interleaving 4 shifted copies in SBUF (Scalar engine) and writing ONE
    contiguous DMA per tile (a strided-write formulation costs 10.5M
    4-byte DMA packets = 32.5ms; this costs 128 x 20KB packets = 0.4ms).
  Phase B: streaming coordinate math (u2, v2, floor, clip, weights) with
    the exact reference op order (bit-exact u2/v2 so floor decisions
    match), then a hardware For_i loop of [128,1]-offset indirect DMA
    gathers (128 descriptors each, the only offset shape the SWDGE
    indirect ucode supports) fetching each pixel's 16-byte quad,
    round-robined over the 4 SWDGE queues, then the 4-tap combine.

The gather is the hard bound: the SWDGE ucode decodes ~1.05us per
indirect instruction (128 descriptors), serial on the Pool engine, so
~2.6M pixel-gathers per core cost ~21.5ms of Pool time. Everything else
is arranged to hide under it: the affine row-planes come from one shared
iota row times per-batch scalars (no per-batch row buffers, so batch 2's
phase A overlaps batch 1's phase B), the combine uses dedicated temps
(so tile t+1's coordinate math overlaps tile t's gathers), and the i32
scratch is sequenced x-side-then-y-side to halve its footprint.

Host does only the O(1) per-batch 3x3 matrix algebra and ships per-batch
scalars as small aux tensors (the NEFF is shared by all cores, so
per-batch constants must arrive as data, not compile-time immediates).
"""

import numpy as np

import concourse.bass as bass
import concourse.bacc as bacc
import concourse.mybir as mybir
from concourse.tile import TileContext
from concourse import bass_utils

B, H, W = 16, 1024, 1280
NCORES = 8
BPC = B // NCORES          # batches per core
HP = H + 1                 # J2 rows
WP = W + 1                 # J2 cols
NTILES = H // 128
UNROLL = 256               # gather columns per For_i iteration

F32 = mybir.dt.float32
I32 = mybir.dt.int32
OP = mybir.AluOpType


def _build_bass():
    nc = bacc.Bacc(target_bir_lowering=False, num_swdge_queues=4)

    d1 = nc.dram_tensor("d1", [BPC, H, W], F32, kind="ExternalInput")
    d2 = nc.dram_tensor("d2", [BPC, H, W], F32, kind="ExternalInput")
    # One shared iota row (x replicated to all partitions), batch-invariant
    urow = nc.dram_tensor("urow", [128, W], F32, kind="ExternalInput")
    # Per-tile per-partition columns [BPC, 128, NTILES]
    colA = nc.dram_tensor("colA", [BPC, 128, NTILES], F32, kind="ExternalInput")  # -M01*y
    colB = nc.dram_tensor("colB", [BPC, 128, NTILES], F32, kind="ExternalInput")  # -M11*y
    colC = nc.dram_tensor("colC", [BPC, 128, NTILES], F32, kind="ExternalInput")  # M21*y
    colG = nc.dram_tensor("colG", [BPC, 128, NTILES], F32, kind="ExternalInput")  # m2_21*y
    # Per-batch scalars replicated to [BPC, 128, 1]
    names = ["cwx", "cwy", "cwz", "cw2", "cA2", "cB2", "cC2", "cG2",
             "sA0", "sB0", "sC0", "sG0"]
    scal = {n: nc.dram_tensor(n, [BPC, 128, 1], F32, kind="ExternalInput")
            for n in names}
    out = nc.dram_tensor("out", [BPC, H, W], F32, kind="ExternalOutput")

    with TileContext(nc) as tc:
        with tc.tile_pool(name="dram", bufs=2, space="DRAM") as dpool, \
             tc.tile_pool(name="cst", bufs=1) as cpool, \
             tc.tile_pool(name="io", bufs=1) as iop, \
             tc.tile_pool(name="tmp", bufs=1) as tp, \
             tc.tile_pool(name="gat", bufs=1) as gp:

            urow_t = cpool.tile([128, W], F32, tag="urow")
            nc.sync.dma_start(out=urow_t[:], in_=urow[:])

            for lb in range(BPC):
                J2 = dpool.tile([HP, WP, 4], F32, tag="J2")

                st = {}
                for n in names:
                    st[n] = cpool.tile([128, 1], F32, tag=n, bufs=2, name=f"st_{n}")
                    nc.sync.dma_start(out=st[n][:], in_=scal[n][lb])
                colA_t = cpool.tile([128, NTILES], F32, tag="colA", bufs=2)
                colB_t = cpool.tile([128, NTILES], F32, tag="colB", bufs=2)
                colC_t = cpool.tile([128, NTILES], F32, tag="colC", bufs=2)
                colG_t = cpool.tile([128, NTILES], F32, tag="colG", bufs=2)
                nc.sync.dma_start(out=colA_t[:], in_=colA[lb])
                nc.sync.dma_start(out=colB_t[:], in_=colB[lb])
                nc.sync.dma_start(out=colC_t[:], in_=colC[lb])
                nc.sync.dma_start(out=colG_t[:], in_=colG[lb])

                def ts(dst, in0, s1, s2, o0, o1=None):
                    nc.vector.tensor_scalar(out=dst, in0=in0, scalar1=s1,
                                            scalar2=s2, op0=o0,
                                            **({"op1": o1} if o1 is not None else {}))

                # ---- Phase A: d1_calc for the whole image into DRAM ----
                d1cD = dpool.tile([H, W], F32, tag="d1cD")
                for t in range(NTILES):
                    y0 = 128 * t
                    d2t = iop.tile([128, W], F32, tag="d2t", bufs=2)
                    nc.sync.dma_start(out=d2t[:], in_=d2[lb, y0:y0 + 128, :])
                    # g = (m2_20*x + m2_21*y) + m2_22  (reference assoc order)
                    g = tp.tile([128, W], F32, tag="g", bufs=2)
                    ts(g[:], urow_t[:], st["sG0"][:, 0:1], None, OP.mult)
                    nc.vector.tensor_tensor(
                        out=g[:], in0=g[:],
                        in1=colG_t[:, t:t + 1].to_broadcast([128, W]), op=OP.add)
                    ts(g[:], g[:], st["cG2"][:, 0:1], None, OP.add)
                    # d1c = d2*g + W2z
                    d1cw = tp.tile([128, W], F32, tag="d1cw", bufs=2)
                    nc.vector.tensor_tensor(
                        out=d1cw[:], in0=d2t[:], in1=g[:], op=OP.mult)
                    ts(d1cw[:], d1cw[:], st["cw2"][:, 0:1], None, OP.add)
                    nc.sync.dma_start(out=d1cD[y0:y0 + 128, :], in_=d1cw[:])

                # J2 row 0: quad = (I[0,cA], I[0,cA], I[0,cB], I[0,cB])
                ld0 = iop.tile([128, W], F32, tag="d2t", bufs=2)
                nc.sync.dma_start(out=ld0[0:1, :], in_=d1cD[0:1, :])
                j2r0 = tp.tile([128, WP, 4], F32, tag="j2sb")
                nc.vector.memset(j2r0[0:1, 0, :], 0.0)
                nc.vector.memset(j2r0[0:1, W, :], 0.0)
                nc.scalar.copy(out=j2r0[0:1, 1:W, 0], in_=ld0[0:1, 0:W - 1])
                nc.scalar.copy(out=j2r0[0:1, 1:W, 1], in_=ld0[0:1, 0:W - 1])
                nc.scalar.copy(out=j2r0[0:1, 1:W, 2], in_=ld0[0:1, 1:W])
                nc.scalar.copy(out=j2r0[0:1, 1:W, 3], in_=ld0[0:1, 1:W])
                nc.sync.dma_start(out=J2[0:1, :, :], in_=j2r0[0:1, :, :])

                # Per tile t: J2 rows 128t+1 .. 128t+128.
                #   k=0,2 from I rows 128t..128t+127  (cur)
                #   k=1,3 from I rows 128t+1..128t+128 (nxt; row H -> H-1)
                for t in range(NTILES):
                    y0 = 128 * t
                    cur = iop.tile([128, W], F32, tag="curt")
                    nc.sync.dma_start(out=cur[:], in_=d1cD[y0:y0 + 128, :])
                    nxt = tp.tile([128, W], F32, tag="nxt")
                    if t < NTILES - 1:
                        nc.sync.dma_start(out=nxt[:], in_=d1cD[y0 + 1:y0 + 129, :])
                    else:
                        nc.sync.dma_start(out=nxt[0:127, :],
                                          in_=d1cD[y0 + 1:y0 + 128, :])
                        # J2 row H: rB = min(H, H-1) = H-1 -> last image row
                        nc.sync.dma_start(out=nxt[127:128, :],
                                          in_=d1cD[H - 1:H, :])
                    j2sb = tp.tile([128, WP, 4], F32, tag="j2sb")
                    nc.vector.memset(j2sb[:, 0, :], 0.0)
                    nc.vector.memset(j2sb[:, W, :], 0.0)
                    nc.scalar.copy(out=j2sb[:, 1:W, 0], in_=cur[:, 0:W - 1])
                    nc.scalar.copy(out=j2sb[:, 1:W, 1], in_=nxt[:, 0:W - 1])
                    nc.scalar.copy(out=j2sb[:, 1:W, 2], in_=cur[:, 1:W])
                    nc.scalar.copy(out=j2sb[:, 1:W, 3], in_=nxt[:, 1:W])
                    nc.sync.dma_start(
                        out=J2[128 * t + 1:128 * t + 129, :, :], in_=j2sb[:])

                J2flat = J2[:].rearrange("a b c -> (a b) c")

                # ---- Phase B ----
                for t in range(NTILES):
                    y0 = 128 * t
                    z1 = iop.tile([128, W], F32, tag="z1")
                    nc.sync.dma_start(out=z1[:], in_=d1[lb, y0:y0 + 128, :])

                    # f32 scratch fa..fe die before the gather; weights live
                    # until the combine (which has its own temps ca/cb so the
                    # next tile's math can overlap this tile's gathers).
                    fa = tp.tile([128, W], F32, tag="fa")
                    fb = tp.tile([128, W], F32, tag="fb")
                    fc = tp.tile([128, W], F32, tag="fc")
                    fd = tp.tile([128, W], F32, tag="fd")
                    fe = tp.tile([128, W], F32, tag="fe")
                    u2 = tp.tile([128, W], F32, tag="u2")
                    v2 = tp.tile([128, W], F32, tag="v2")
                    # A = (-M00*x + -M01*y) + -M02, reference assoc order
                    ts(fa[:], urow_t[:], st["sA0"][:, 0:1], None, OP.mult)
                    nc.vector.tensor_tensor(
                        out=fa[:], in0=fa[:],
                        in1=colA_t[:, t:t + 1].to_broadcast([128, W]), op=OP.add)
                    ts(fa[:], fa[:], st["cA2"][:, 0:1], None, OP.add)
                    ts(fb[:], urow_t[:], st["sB0"][:, 0:1], None, OP.mult)
                    nc.vector.tensor_tensor(
                        out=fb[:], in0=fb[:],
                        in1=colB_t[:, t:t + 1].to_broadcast([128, W]), op=OP.add)
                    ts(fb[:], fb[:], st["cB2"][:, 0:1], None, OP.add)
                    ts(fc[:], urow_t[:], st["sC0"][:, 0:1], None, OP.mult)
                    nc.vector.tensor_tensor(
                        out=fc[:], in0=fc[:],
                        in1=colC_t[:, t:t + 1].to_broadcast([128, W]), op=OP.add)
                    ts(fc[:], fc[:], st["cC2"][:, 0:1], None, OP.add)
                    # z2 = z1*C + Wv2
                    zd = tp.tile([128, W], F32, tag="zd")
                    nc.vector.tensor_tensor(out=zd[:], in0=z1[:], in1=fc[:], op=OP.mult)
                    ts(zd[:], zd[:], st["cwz"][:, 0:1], None, OP.add)
                    nc.vector.reciprocal(out=fe[:], in_=zd[:])          # r0
                    nc.vector.tensor_tensor(out=fd[:], in0=zd[:], in1=fe[:], op=OP.mult)
                    ts(fd[:], fd[:], 2.0, None, OP.subtract)            # z2*r0-2
                    nc.vector.tensor_tensor(out=fe[:], in0=fe[:], in1=fd[:], op=OP.mult)  # q=-1/z2
                    # nU = z1*A + (-Wv0)  (A,Wv negated on host); u2' = nU*q
                    nU = tp.tile([128, W], F32, tag="nU")
                    nc.vector.tensor_tensor(out=nU[:], in0=z1[:], in1=fa[:], op=OP.mult)
                    ts(nU[:], nU[:], st["cwx"][:, 0:1], None, OP.add)
                    nc.vector.tensor_tensor(out=u2[:], in0=nU[:], in1=fe[:], op=OP.mult)
                    # quotient correction: e = z2*u2' + nU; u2 = u2' + e*q
                    nc.vector.tensor_tensor(out=fd[:], in0=zd[:], in1=u2[:], op=OP.mult)
                    nc.vector.tensor_tensor(out=fd[:], in0=fd[:], in1=nU[:], op=OP.add)
                    nc.vector.tensor_tensor(out=fd[:], in0=fd[:], in1=fe[:], op=OP.mult)
                    nc.vector.tensor_tensor(out=u2[:], in0=u2[:], in1=fd[:], op=OP.add)
                    nc.vector.tensor_tensor(out=nU[:], in0=z1[:], in1=fb[:], op=OP.mult)
                    ts(nU[:], nU[:], st["cwy"][:, 0:1], None, OP.add)
                    nc.vector.tensor_tensor(out=v2[:], in0=nU[:], in1=fe[:], op=OP.mult)
                    nc.vector.tensor_tensor(out=fd[:], in0=zd[:], in1=v2[:], op=OP.mult)
                    nc.vector.tensor_tensor(out=fd[:], in0=fd[:], in1=nU[:], op=OP.add)
                    nc.vector.tensor_tensor(out=fd[:], in0=fd[:], in1=fe[:], op=OP.mult)
                    nc.vector.tensor_tensor(out=v2[:], in0=v2[:], in1=fd[:], op=OP.add)

                    wxa = tp.tile([128, W], F32, tag="wxa")
                    wxc = tp.tile([128, W], F32, tag="wxc")
                    wya = tp.tile([128, W], F32, tag="wya")
                    wyb = tp.tile([128, W], F32, tag="wyb")
                    ia = tp.tile([128, W], I32, tag="ia")
                    ib2 = tp.tile([128, W], I32, tag="ib2")
                    icc = tp.tile([128, W], I32, tag="icc")
                    idt = tp.tile([128, W], I32, tag="idt")
                    flat = tp.tile([128, W], I32, tag="flat")

                    # --- x side: exact floor, clips, weights ---
                    # floor: t = cvt(u2); t -= (cvt_f32(t) > u2)
                    nc.vector.tensor_copy(out=ia[:], in_=u2[:])
                    nc.vector.tensor_copy(out=fa[:], in_=ia[:])
                    nc.vector.tensor_tensor(out=fc[:], in0=fa[:], in1=u2[:], op=OP.is_gt)
                    nc.vector.tensor_copy(out=idt[:], in_=fc[:])
                    nc.vector.tensor_tensor(out=ia[:], in0=ia[:], in1=idt[:], op=OP.subtract)
                    # m1 = max(x0+1,0); x1c = min(m1,W-1); cc = min(m1,W); x0c
                    ts(idt[:], ia[:], 1, 0, OP.add, OP.max)
                    ts(ib2[:], idt[:], W - 1, None, OP.min)             # x1c
                    ts(icc[:], idt[:], W, None, OP.min)                 # cc (to flat)
                    ts(ia[:], ia[:], 0, W - 1, OP.max, OP.min)          # x0c in place
                    nc.vector.tensor_copy(out=fa[:], in_=ib2[:])        # x1f
                    nc.vector.tensor_tensor(out=wxa[:], in0=fa[:], in1=u2[:], op=OP.subtract)
                    nc.vector.tensor_copy(out=fa[:], in_=ia[:])         # x0f
                    nc.vector.tensor_tensor(out=wxc[:], in0=u2[:], in1=fa[:], op=OP.subtract)

                    # --- y side (reuses ia/ib2/idt) ---
                    nc.vector.tensor_copy(out=ia[:], in_=v2[:])
                    nc.vector.tensor_copy(out=fb[:], in_=ia[:])
                    nc.vector.tensor_tensor(out=fc[:], in0=fb[:], in1=v2[:], op=OP.is_gt)
                    nc.vector.tensor_copy(out=idt[:], in_=fc[:])
                    nc.vector.tensor_tensor(out=ia[:], in0=ia[:], in1=idt[:], op=OP.subtract)
                    ts(idt[:], ia[:], 1, 0, OP.add, OP.max)
                    ts(ib2[:], idt[:], H, None, OP.min)                 # rr (to flat)
                    ts(idt[:], idt[:], H - 1, None, OP.min)             # y1c in place
                    ts(ia[:], ia[:], 0, H - 1, OP.max, OP.min)          # y0c
                    nc.vector.tensor_copy(out=fb[:], in_=idt[:])        # y1f
                    nc.vector.tensor_tensor(out=wya[:], in0=fb[:], in1=v2[:], op=OP.subtract)
                    nc.vector.tensor_copy(out=fb[:], in_=ia[:])         # y0f
                    nc.vector.tensor_tensor(out=wyb[:], in0=v2[:], in1=fb[:], op=OP.subtract)
                    nc.vector.scalar_tensor_tensor(
                        out=flat[:], in0=ib2[:], scalar=WP, in1=icc[:],
                        op0=OP.mult, op1=OP.add)

                    # gather loop: 128 descriptors per indirect DMA
                    gq = gp.tile([128, W, 4], F32, tag="gq")
                    ib = gp.tile([128, UNROLL], I32, tag="ib", bufs=2)
                    gb = gp.tile([128, UNROLL, 4], F32, tag="gb", bufs=2)
                    with tc.For_i(0, W, UNROLL) as iv:
                        nc.vector.tensor_copy(out=ib[:], in_=flat[:, bass.ds(iv, UNROLL)])
                        for j in range(UNROLL):
                            inst = nc.gpsimd.indirect_dma_start(
                                out=gb[:, j, :], out_offset=None,
                                in_=J2flat,
                                in_offset=bass.IndirectOffsetOnAxis(ap=ib[:, j:j + 1], axis=0),
                            )
                            inst.ins.queue = f"qPoolDynamic{(j // 4) % 4 or ''}"
                        nc.vector.tensor_copy(out=gq[:, bass.ds(iv, UNROLL), :], in_=gb[:])

                    # combine, reference order:
                    # ((wa*Ia + wb*Ib) + wc*Ic) + wd*Id, wa = wxa*wya etc.
                    # Dedicated temps ca/cb + in-place weight products keep
                    # fa..fe free so tile t+1's math overlaps these ops.
                    ca = tp.tile([128, W], F32, tag="ca")
                    cb = tp.tile([128, W], F32, tag="cb")
                    ot = iop.tile([128, W], F32, tag="ot")
                    nc.vector.tensor_tensor(out=ca[:], in0=wxa[:], in1=wyb[:], op=OP.mult)   # wb
                    nc.vector.tensor_tensor(out=wxa[:], in0=wxa[:], in1=wya[:], op=OP.mult)  # wa
                    nc.vector.tensor_tensor(out=cb[:], in0=wxc[:], in1=wya[:], op=OP.mult)   # wc
                    nc.vector.tensor_tensor(out=wxc[:], in0=wxc[:], in1=wyb[:], op=OP.mult)  # wd
                    nc.vector.tensor_tensor(out=ot[:], in0=wxa[:], in1=gq[:, :, 0], op=OP.mult)
                    nc.vector.tensor_tensor(out=wxa[:], in0=ca[:], in1=gq[:, :, 1], op=OP.mult)
                    nc.vector.tensor_tensor(out=ot[:], in0=ot[:], in1=wxa[:], op=OP.add)
                    nc.vector.tensor_tensor(out=wxa[:], in0=cb[:], in1=gq[:, :, 2], op=OP.mult)
                    nc.vector.tensor_tensor(out=ot[:], in0=ot[:], in1=wxa[:], op=OP.add)
                    nc.vector.tensor_tensor(out=wxa[:], in0=wxc[:], in1=gq[:, :, 3], op=OP.mult)
                    nc.vector.tensor_tensor(out=ot[:], in0=ot[:], in1=wxa[:], op=OP.add)
                    nc.sync.dma_start(out=out[lb, y0:y0 + 128, :], in_=ot[:])

    nc.finalize()
    return nc


def _host_aux(translation, rotation, intrinsic):
    """Per-batch coefficient tensors (f32, mirroring reference order of ops)."""
    K = intrinsic.astype(np.float32)
    Kinv = np.linalg.inv(K).astype(np.float32)
    R = rotation.astype(np.float32)
    t = translation.astype(np.float32)
    nb = R.shape[0]
    temp = np.einsum('ij,bkj->bik', K, R).astype(np.float32)
    Wv = np.einsum('bij,bjk->bik', temp, -t).astype(np.float32)     # [nb,3,1]
    M = np.einsum('bij,jk->bik', temp, Kinv).astype(np.float32)     # [nb,3,3]
    W2 = np.einsum('ij,bjk->bik', K, t).astype(np.float32)
    M2 = np.einsum('bij,jk->bik', np.einsum('ij,bjk->bik', K, R), Kinv).astype(np.float32)

    y = np.arange(H, dtype=np.float32)
    ycols = y.reshape(NTILES, 128).T                                # [128, NTILES]

    aux = {}
    aux["colA"] = -(M[:, 0, 1][:, None, None] * ycols[None])
    aux["colB"] = -(M[:, 1, 1][:, None, None] * ycols[None])
    aux["colC"] = (M[:, 2, 1][:, None, None] * ycols[None])
    aux["colG"] = (M2[:, 2, 1][:, None, None] * ycols[None])
    ones = np.ones((nb, 128, 1), np.float32)
    aux["cwx"] = -Wv[:, 0, 0][:, None, None] * ones
    aux["cwy"] = -Wv[:, 1, 0][:, None, None] * ones
    aux["cwz"] = Wv[:, 2, 0][:, None, None] * ones
    aux["cw2"] = W2[:, 2, 0][:, None, None] * ones
    aux["cA2"] = -M[:, 0, 2][:, None, None] * ones
    aux["cB2"] = -M[:, 1, 2][:, None, None] * ones
    aux["cC2"] = M[:, 2, 2][:, None, None] * ones
    aux["cG2"] = M2[:, 2, 2][:, None, None] * ones
    aux["sA0"] = -M[:, 0, 0][:, None, None] * ones
    aux["sB0"] = -M[:, 1, 0][:, None, None] * ones
    aux["sC0"] = M[:, 2, 0][:, None, None] * ones
    aux["sG0"] = M2[:, 2, 0][:, None, None] * ones
    for k in aux:
        aux[k] = np.ascontiguousarray(aux[k].astype(np.float32))
    return aux


_NC_CACHE = {}


def kernel(depth_map_1, depth_map_2, translation, rotation, intrinsic):
    d1 = np.ascontiguousarray(np.asarray(depth_map_1, dtype=np.float32)[..., 0])
    d2 = np.ascontiguousarray(np.asarray(depth_map_2, dtype=np.float32)[..., 0])
    t = np.asarray(translation, dtype=np.float32)
    R = np.asarray(rotation, dtype=np.float32)
    K = np.asarray(intrinsic, dtype=np.float32)

    if "nc" not in _NC_CACHE:
        _NC_CACHE["nc"] = _build_bass()
    nc = _NC_CACHE["nc"]

    aux = _host_aux(t, R, K)
    urow = np.ascontiguousarray(
        np.tile(np.arange(W, dtype=np.float32)[None, :], (128, 1)))

    in_maps = []
    for c in range(NCORES):
        sl = slice(c * BPC, (c + 1) * BPC)
        m = {"d1": d1[sl], "d2": d2[sl], "urow": urow}
        for k, v in aux.items():
            m[k] = v[sl]
        in_maps.append(m)

    res = bass_utils.run_bass_kernel_spmd(nc, in_maps, core_ids=list(range(NCORES)))
    out = np.empty((B, H, W, 1), np.float32)
    for c in range(NCORES):
        out[c * BPC:(c + 1) * BPC, :, :, 0] = res.results[c]["out"]
    return out
